# revision 13
# baseline (speedup 1.0000x reference)
"""AnomalyAttention Trainium2 kernel: 8-core data-parallel (batch sharded).

Computes, per the reference:
  V      (B,L,H,D)   = series @ values
  series (B,H,L,L)   = softmax(scores/8) with zeroed diagonal
  prior  (B,H,L,L)   = gaussian(distances; sig)
  sig    (B,H,L,L)   = broadcast of transformed sigma
  M      (B,H,L)     = (max-mean)(scores) - (max-mean')(band-masked scores)
"""
import math
import sys

sys.path.insert(0, "/opt/trn_rl_repo")

import numpy as np
from contextlib import ExitStack

import concourse.bass as bass
import concourse.tile as tile
from concourse import bacc, mybir
from concourse.bass import ts
from concourse.bass_utils import run_bass_kernel_spmd
from concourse.masks import make_identity

B, L, H, E, D = 256, 100, 8, 64, 64
NCORES = 8
BS = B // NCORES  # 32 batches per core
F32 = mybir.dt.float32
AX = mybir.AxisListType
AF = mybir.ActivationFunctionType

MASK_DOT = 3          # |i-j| < 3 zeroed in 'sparse' scores
SCALE = 1.0 / math.sqrt(E)
LN2PI_HALF = 0.5 * math.log(2.0 * math.pi)

# toggle: emit sig output via broadcast DMA (0-stride source) vs DVE compute
USE_SIG_DMA_BCAST = False


def _build_nc(nb=BS, feat=0xFF, stop=99):
    nc = bacc.Bacc()
    q_d = nc.dram_tensor("queries", [BS, L, H, E], F32, kind="ExternalInput")
    k_d = nc.dram_tensor("keys", [BS, L, H, E], F32, kind="ExternalInput")
    v_d = nc.dram_tensor("values", [BS, L, H, D], F32, kind="ExternalInput")
    sg_d = nc.dram_tensor("sigma", [BS, L, H], F32, kind="ExternalInput")
    di_d = nc.dram_tensor("distances", [L, L], F32, kind="ExternalInput")

    vo_d = nc.dram_tensor("v_out", [BS, L, H, D], F32, kind="ExternalOutput")
    se_d = nc.dram_tensor("series_out", [BS, H, L, L], F32, kind="ExternalOutput")
    pr_d = nc.dram_tensor("prior_out", [BS, H, L, L], F32, kind="ExternalOutput")
    si_d = nc.dram_tensor("sig_out", [BS, H, L, L], F32, kind="ExternalOutput")
    m_d = nc.dram_tensor("m_out", [BS, H, L], F32, kind="ExternalOutput")

    with ExitStack() as ctx:
        tc = ctx.enter_context(tile.TileContext(nc))
        singles = ctx.enter_context(tc.tile_pool(name="singles", bufs=1))
        io_pool = ctx.enter_context(tc.tile_pool(name="io", bufs=3))
        qt_pool = ctx.enter_context(tc.tile_pool(name="qt", bufs=2))
        sp_pool = ctx.enter_context(tc.tile_pool(name="sp", bufs=2))
        ex_pool = ctx.enter_context(tc.tile_pool(name="ex", bufs=2))
        ser_pool = ctx.enter_context(tc.tile_pool(name="ser", bufs=2))
        pr_pool = ctx.enter_context(tc.tile_pool(name="pr", bufs=2))
        st_pool = ctx.enter_context(tc.tile_pool(name="st", bufs=2))
        vo_pool = ctx.enter_context(tc.tile_pool(name="vo", bufs=2))
        stat_pool = ctx.enter_context(tc.tile_pool(name="stat", bufs=3))
        mt_pool = ctx.enter_context(tc.tile_pool(name="mt", bufs=2))

        tp_psum = ctx.enter_context(tc.tile_pool(name="tp_ps", bufs=2, space="PSUM"))
        sc_psum = ctx.enter_context(tc.tile_pool(name="sc_ps", bufs=2, space="PSUM"))
        tv_psum = ctx.enter_context(tc.tile_pool(name="tv_ps", bufs=2, space="PSUM"))
        vo_psum = ctx.enter_context(tc.tile_pool(name="vo_ps", bufs=2, space="PSUM"))

        # ---------------- one-time setup ----------------
        ident = singles.tile([128, 128], F32)
        make_identity(nc, ident)

        # distances^2
        dist_sb = singles.tile([L, L], F32)
        nc.sync.dma_start(out=dist_sb, in_=di_d[:, :])
        dist2_sb = singles.tile([L, L], F32)
        nc.vector.tensor_mul(dist2_sb, dist_sb, dist_sb)

        # band mask, replicated for 4 heads: 0 where |s-l| <= 2 else 1
        band_sb = singles.tile([L, 4, L], F32)
        nc.vector.memset(band_sb, 1.0)
        # keep 1 where s-l-3 >= 0, else 0  (zeroes band + left of band)
        nc.gpsimd.affine_select(
            out=band_sb, in_=band_sb, compare_op=mybir.AluOpType.is_ge,
            fill=0.0, base=-MASK_DOT, channel_multiplier=-1,
            pattern=[[0, 4], [1, L]],
        )
        # keep current where s-l+2 >= 0, fill 1 where s < l-2 (restore left)
        nc.gpsimd.affine_select(
            out=band_sb, in_=band_sb, compare_op=mybir.AluOpType.is_ge,
            fill=1.0, base=MASK_DOT - 1, channel_multiplier=-1,
            pattern=[[0, 4], [1, L]],
        )

        # sigma-derived per-(l,b,h) scalars
        nf = BS * H
        sig_all = singles.tile([L, BS, H], F32)
        nc.sync.dma_start(out=sig_all, in_=sg_d[:, :, :].rearrange("b l h -> l b h"))
        sig_flat = sig_all.rearrange("p b h -> p (b h)")
        t1 = singles.tile([L, nf], F32)
        # sigmoid(5x) = 0.5 + 0.5*tanh(2.5x); +1e-5
        nc.scalar.activation(t1, sig_flat, AF.Tanh, scale=2.5)
        sgm = singles.tile([L, nf], F32)
        nc.vector.tensor_scalar(
            out=sgm, in0=t1, scalar1=0.5, scalar2=0.5 + 1e-5,
            op0=mybir.AluOpType.mult, op1=mybir.AluOpType.add,
        )
        # sig = 3^sgm - 1 = exp(sgm*ln3) - 1
        p3 = singles.tile([L, nf], F32)
        nc.scalar.activation(p3, sgm, AF.Exp, scale=math.log(3.0))
        sig_val = singles.tile([L, nf], F32)
        nc.vector.tensor_scalar_add(sig_val, p3, -1.0)
        # ln(sig)
        lnsig = singles.tile([L, nf], F32)
        nc.scalar.activation(lnsig, sig_val, AF.Ln)
        # coefln = -ln(sig) - 0.5*ln(2*pi)   [prior exp bias]
        coefln = singles.tile([L, nf], F32)
        nc.vector.tensor_scalar(
            out=coefln, in0=lnsig, scalar1=-1.0, scalar2=-LN2PI_HALF,
            op0=mybir.AluOpType.mult, op1=mybir.AluOpType.add,
        )
        # invsig2m = -0.5 * exp(-2*ln(sig)) = -0.5/sig^2  [prior exp scale]
        iv2 = singles.tile([L, nf], F32)
        nc.scalar.activation(iv2, lnsig, AF.Exp, scale=-2.0)
        invsig2m = singles.tile([L, nf], F32)
        nc.vector.tensor_scalar_mul(invsig2m, iv2, -0.5)

        sig_val3 = sig_val.rearrange("p (b h) -> p b h", b=BS)
        coefln3 = coefln.rearrange("p (b h) -> p b h", b=BS)
        invsig2m3 = invsig2m.rearrange("p (b h) -> p b h", b=BS)

        ones_sb = None
        if not USE_SIG_DMA_BCAST:
            ones_sb = singles.tile([L, L], F32)
            nc.vector.memset(ones_sb, 1.0)

        # ---------------- per-batch pipeline ----------------
        for b in range(nb):
            q_sb = io_pool.tile([L, H, E], F32, tag="q")
            k_sb = io_pool.tile([L, H, E], F32, tag="k")
            v_sb = io_pool.tile([L, H, D], F32, tag="v")
            nc.sync.dma_start(out=q_sb, in_=q_d[b])
            nc.sync.dma_start(out=k_sb, in_=k_d[b])
            nc.sync.dma_start(out=v_sb, in_=v_d[b])
            if stop <= 0:
                continue

            # transpose Q,K per head: (100,64) -> (64,100), all at base 0
            qT_sb = qt_pool.tile([64, H, L], F32, tag="qT")
            kT_sb = qt_pool.tile([64, H, L + 1], F32, tag="kT")
            for g4 in range(2):
                tpq = tp_psum.tile([64, 4, L], F32, tag="tp")
                for hh in range(4):
                    h = g4 * 4 + hh
                    nc.tensor.transpose(tpq[:, hh, :], q_sb[:, h, :],
                                        ident[:L, :L])
                nc.vector.tensor_copy(qT_sb[:, ts(g4, 4), :], tpq)
                tpk = tp_psum.tile([64, 4, L], F32, tag="tp")
                for hh in range(4):
                    h = g4 * 4 + hh
                    nc.tensor.transpose(tpk[:, hh, :], k_sb[:, h, :],
                                        ident[:L, :L])
                nc.vector.tensor_copy(kT_sb[:, ts(g4, 4), :L], tpk)
            # ksum column: rowsum(scores) = Q @ ksum rides as rhs col 100
            nc.vector.reduce_sum(kT_sb[:, :, L:L + 1], kT_sb[:, :, :L], axis=AX.X)
            if stop <= 1:
                continue

            ex_sb = ex_pool.tile([L, H, L], F32, tag="ex")
            ser_sb = ser_pool.tile([L, H, L], F32, tag="ser")
            mx = stat_pool.tile([L, H], F32, tag="mx")
            sm = stat_pool.tile([L, H], F32, tag="sm")
            smx = stat_pool.tile([L, H], F32, tag="smx")
            ssm = stat_pool.tile([L, H], F32, tag="ssm")
            esum = stat_pool.tile([L, H], F32, tag="esum")

            for g in range(2):
                scp = sc_psum.tile([L, 4, L + 1], F32, tag="sc")
                for hh in range(4):
                    h = g * 4 + hh
                    nc.tensor.matmul(
                        scp[:, hh, :],
                        lhsT=qT_sb[:, h, :],
                        rhs=kT_sb[:, h, :],
                        start=True, stop=True,
                    )
                gs = slice(g * 4, g * 4 + 4)
                if stop > 3:
                    # stats on raw scores
                    nc.vector.reduce_max(mx[:, gs], scp[:, :, :L], axis=AX.X)
                    nc.vector.tensor_copy(sm[:, gs], scp[:, :, L:L + 1])
                if stop > 4:
                    # band-masked scores
                    sp_sb = sp_pool.tile([L, 4, L], F32, tag="spr")
                    nc.vector.tensor_mul(sp_sb, scp[:, :, :L], band_sb)
                    nc.vector.reduce_max(smx[:, gs], sp_sb, axis=AX.X)
                    nc.vector.reduce_sum(ssm[:, gs], sp_sb, axis=AX.X)
                if stop > 5:
                    # softmax numerator exp(scores/8) + per-head row sums
                    for hh in range(4):
                        h = g * 4 + hh
                        nc.scalar.activation(
                            ex_sb[:, h, :], scp[:, hh, :L], AF.Exp,
                            scale=SCALE, accum_out=esum[:, h:h + 1],
                        )

            if stop <= 6:
                continue
            # series = exp * (1/esum), diagonal zeroed
            inv = stat_pool.tile([L, H], F32, tag="inv")
            nc.vector.reciprocal(inv, esum)
            for h in range(H):
                nc.vector.tensor_scalar_mul(ser_sb[:, h, :], ex_sb[:, h, :],
                                            inv[:, h:h + 1])
            nc.gpsimd.affine_select(
                out=ser_sb, in_=ser_sb, compare_op=mybir.AluOpType.not_equal,
                fill=0.0, base=0, channel_multiplier=-1,
                pattern=[[0, H], [1, L]],
            )

            # M = (mx - sm/100) - (smx - ssm/94)
            if feat & 2:
             m1 = stat_pool.tile([L, H], F32, tag="m1")
             m2 = stat_pool.tile([L, H], F32, tag="m2")
             mres = stat_pool.tile([L, H], F32, tag="mres")
             nc.vector.tensor_scalar_mul(m1, sm, 1.0 / L)
             nc.vector.tensor_sub(m1, mx, m1)
             nc.vector.tensor_scalar_mul(m2, ssm, 1.0 / (L - 2 * MASK_DOT))
             nc.vector.tensor_sub(m2, smx, m2)
             nc.vector.tensor_sub(mres, m1, m2)
             # transpose (100,8) -> (8,100) for contiguous DMA
             mt_ps = tv_psum.tile([H, L], F32, tag="tv")
             nc.tensor.transpose(mt_ps, mres, ident[:L, :L])
             mt_sb = mt_pool.tile([H, L], F32, tag="mt")
             nc.vector.tensor_copy(mt_sb, mt_ps)
             nc.sync.dma_start(out=m_d[b], in_=mt_sb)

            # prior = exp(dist2 * (-0.5/sig^2) + coefln)  (one ACT op per head)
            if feat & 4:
             pr_sb = pr_pool.tile([L, H, L], F32, tag="pr")
             for h in range(H):
                nc.scalar.activation(
                    pr_sb[:, h, :], dist2_sb, AF.Exp,
                    scale=invsig2m3[:, b, h:h + 1], bias=coefln3[:, b, h:h + 1],
                )
             nc.sync.dma_start(out=pr_d[b].rearrange("h l s -> l h s"), in_=pr_sb)

            # sig output: broadcast sig_val along s
            if (feat & 8) and USE_SIG_DMA_BCAST:
                sv = sig_val3[:, b, :]
                bcast = bass.AP(tensor=sv.tensor, offset=sv.offset,
                                ap=[sv.ap[0], sv.ap[1], [0, L]])
                nc.gpsimd.dma_start(out=si_d[b].rearrange("h l s -> l h s"),
                                    in_=bcast)
            elif feat & 8:
                sig_sb = pr_pool.tile([L, H, L], F32, tag="sig")
                for h in range(H):
                    nc.vector.tensor_scalar_mul(sig_sb[:, h, :], ones_sb,
                                                sig_val3[:, b, h:h + 1])
                nc.sync.dma_start(out=si_d[b].rearrange("h l s -> l h s"),
                                  in_=sig_sb)

            # V = series @ values : transpose series, then contract over s
            if feat & 16:
             vo_sb = vo_pool.tile([L, H, D], F32, tag="vo")
             for g in range(2):
                vps = vo_psum.tile([L, 4, D], F32, tag="vps")
                for j in range(2):
                    tv = tv_psum.tile([L, 2, L], F32, tag="tv")
                    for i in range(2):
                        h = g * 4 + j * 2 + i
                        nc.tensor.transpose(tv[:, i, :], ser_sb[:, h, :],
                                            ident[:L, :L])
                    sT_sb = st_pool.tile([L, 2, L], F32, tag="sT")
                    nc.vector.tensor_copy(sT_sb, tv)
                    for i in range(2):
                        h = g * 4 + j * 2 + i
                        nc.tensor.matmul(
                            vps[:, j * 2 + i, :],
                            lhsT=sT_sb[:, i, :],
                            rhs=v_sb[:, h, :],
                            start=True, stop=True,
                        )
                nc.vector.tensor_copy(vo_sb[:, ts(g, 4), :], vps)
             nc.sync.dma_start(out=vo_d[b], in_=vo_sb)
            if feat & 1:
             nc.sync.dma_start(out=se_d[b].rearrange("h l s -> l h s"), in_=ser_sb)

    nc.finalize()
    return nc


_CACHE = {}


def _get_nc():
    if "nc" not in _CACHE:
        _CACHE["nc"] = _build_nc()
    return _CACHE["nc"]


def _install_profile_hook():
    """Synthesize antenv.axon_hooks (absent in this image) and register the
    ctypes NTFF hook so run_bass_kernel_spmd(trace=True) yields exec_time_ns."""
    import types
    import antenv
    import concourse.bass_utils as bu

    if "antenv.axon_hooks" not in sys.modules:
        mod = types.ModuleType("antenv.axon_hooks")
        mod._hook = None
        mod.set_axon_ntff_profile_hook = lambda h: setattr(mod, "_hook", h)
        mod.get_axon_ntff_profile_hook = lambda: mod._hook
        sys.modules["antenv.axon_hooks"] = mod
        antenv.axon_hooks = mod
        sys.path.insert(0, "/root/.axon_site")
        from trn_agent_boot.trn_boot import _ntff_profile_via_ctypes
        mod.set_axon_ntff_profile_hook(
            _ntff_profile_via_ctypes("/opt/axon/libaxon_pjrt.so"))
    bu.upload_artifacts = lambda tmpdir: tmpdir


def kernel(queries, keys, values, sigma, distances, attn_mask=None, **_):
    nc = _get_nc()
    qs = np.ascontiguousarray(np.asarray(queries, dtype=np.float32))
    ks = np.ascontiguousarray(np.asarray(keys, dtype=np.float32))
    vs = np.ascontiguousarray(np.asarray(values, dtype=np.float32))
    sg = np.ascontiguousarray(np.asarray(sigma, dtype=np.float32))
    di = np.ascontiguousarray(np.asarray(distances, dtype=np.float32))

    in_maps = []
    for c in range(NCORES):
        sl = slice(c * BS, (c + 1) * BS)
        in_maps.append({
            "queries": qs[sl], "keys": ks[sl], "values": vs[sl],
            "sigma": sg[sl], "distances": di,
        })

    import os
    trace = bool(int(os.environ.get("KERNEL_TRACE", "0")))
    if trace:
        _install_profile_hook()
    res = run_bass_kernel_spmd(nc, in_maps, core_ids=list(range(NCORES)),
                               trace=trace,
                               tmpdir=os.environ.get("KERNEL_TRACE_DIR"))
    outs = res.results
    _CACHE["exec_time_ns"] = res.exec_time_ns
    V = np.concatenate([o["v_out"] for o in outs], axis=0)
    series = np.concatenate([o["series_out"] for o in outs], axis=0)
    prior = np.concatenate([o["prior_out"] for o in outs], axis=0)
    sig = np.concatenate([o["sig_out"] for o in outs], axis=0)
    M = np.concatenate([o["m_out"] for o in outs], axis=0)
    return V, series, prior, sig, M
